# revision 4
# baseline (speedup 1.0000x reference)
"""Trainium2 Bass kernel for nn_AudioDeviceModel (dilated causal conv stack).

Strategy (v2 — bf16 + time-folding):
  - Data parallel: batch 64 sharded as 8 rows per core across 8 cores.
  - Only the last FRAME=128 timesteps are output; receptive field 2047, so
    only the last 2174 input samples matter.  Per-layer output windows W_Y
    shrink accordingly.
  - All matmul operands are bf16.  (float32r lowers to fp32_mode=HIGH on
    this toolchain: 4 cycles/row + a 219ns weight reload per matmul — bf16
    runs at 1 cycle/row.)  PSUM accumulation and the h residual chain stay
    fp32, so only activation/weight rounding (~2^-9) enters per layer.
  - Time folding: each folded layer splits its output window into two
    halves stacked on partitions (rows 0:64 = (b,c) of half 1, rows 64:128
    = half 2) with block-diagonal weights kron(eye16, W).  Each conv tap is
    one K=128 matmul over W_Y/2 columns (3 taps, no dup tensors), the 1x1
    residual is one K=128 matmul.  PE cost 2*W_Y cycles/layer vs 3*W_Y for
    tap-stacking, and every DVE/ACT op covers 128 partitions.
  - Layer 0 folds all 3 taps AND the two halves into one K=48 matmul from a
    shifted-triplicated x tile (XS3); its residual broadcast of x reuses
    XS3 rows 32:48 via a ones block-matrix.  Layer 9 (only 128 output cols)
    runs unfolded with K=64 taps.
  - Epilogue per layer: DVE tensor_adds produce H_{i+1} = ph + H_i in fp32
    (fold-i layout, PSUM+SBUF reads); gpsimd (Pool) re-folds H_{i+1} into
    the next layer's A bf16 tensor (SBUF-only copies); ACT does relu+bias.
    Small boundary-crossing pieces go through a scratch copy so every
    tensor_tensor keeps matching partition bases.
  - Mixer: 10 accumulated [64,8]x[64,128] bf16 matmuls interleaved at each
    layer's end (skip_group_check) + bias.
"""

import sys

import numpy as np
import ml_dtypes

try:
    import concourse.bass as bass
except ImportError:  # fresh environment without the site path
    sys.path.insert(0, "/opt/trn_rl_repo")
    import concourse.bass as bass

import concourse.tile as tile
from concourse import bacc, mybir
from concourse.bass_utils import run_bass_kernel_spmd

N_LAYERS = 10
FRAME = 128
B, T = 64, 4096
N_CORES = 8
B_LOC = B // N_CORES  # 8 batch rows per core
NT = 512  # time-tile (one PSUM bank of f32)

# per-layer dilations and windows
DIL = [2**i for i in range(N_LAYERS)]
W_Y = [0] * N_LAYERS  # output window of layer i
W_H = [0] * N_LAYERS  # input window of layer i
W_Y[N_LAYERS - 1] = FRAME
for _i in range(N_LAYERS - 1, -1, -1):
    W_H[_i] = W_Y[_i] + 2 * DIL[_i]
    if _i > 0:
        W_Y[_i - 1] = W_H[_i]
W_X = W_H[0]  # 2174
HW = [w // 2 for w in W_Y]  # folded half-width (layers 0..8)

_F32 = mybir.dt.float32
_F16 = mybir.dt.float16
_RELU = mybir.ActivationFunctionType.Relu
_IDENT = mybir.ActivationFunctionType.Identity


def _tiles(wy):
    """End-aligned tiling: ragged first tile, then 512-wide tiles."""
    r = wy % NT
    starts = ([0] if r else []) + list(range(r, wy, NT))
    return [(s, (starts[k + 1] if k + 1 < len(starts) else wy) - s)
            for k, s in enumerate(starts)]


def _build_program():
    nc = bacc.Bacc(
        "TRN2",
        target_bir_lowering=False,
        debug=False,
        enable_asserts=True,
        num_devices=N_CORES,
    )

    d_x = nc.dram_tensor("xw", [B_LOC, W_X], _F16, kind="ExternalInput").ap()
    d_w0 = nc.dram_tensor("w0", [48, 128], _F16, kind="ExternalInput").ap()
    d_wc = nc.dram_tensor("wc", [128, 8 * 3 * 128], _F16, kind="ExternalInput").ap()
    d_w9 = nc.dram_tensor("w9", [64, 192], _F16, kind="ExternalInput").ap()
    d_wr = nc.dram_tensor("wr", [128, 9 * 128], _F16, kind="ExternalInput").ap()
    d_xb = nc.dram_tensor("xb", [16, 128], _F16, kind="ExternalInput").ap()
    d_wm = nc.dram_tensor("wm", [64, 80], _F16, kind="ExternalInput").ap()
    d_wm9 = nc.dram_tensor("wm9", [64, 8], _F16, kind="ExternalInput").ap()
    d_cb = nc.dram_tensor("cb", [128, N_LAYERS], _F32, kind="ExternalInput").ap()
    d_mb = nc.dram_tensor("mb", [8, 1], _F32, kind="ExternalInput").ap()
    d_out = nc.dram_tensor("out", [B_LOC, FRAME], _F32, kind="ExternalOutput").ap()

    with tile.TileContext(nc) as tc:
        with (
            tc.tile_pool(name="wpool", bufs=1) as wpool,
            tc.tile_pool(name="apool", bufs=2) as apool,
            tc.tile_pool(name="hpool", bufs=2) as hpool,
            tc.tile_pool(name="ypool", bufs=4) as ypool,
            tc.tile_pool(name="spool", bufs=2) as spool,
            tc.tile_pool(name="opool", bufs=1) as opool,
            tc.tile_pool(name="py", bufs=3, space="PSUM") as pyp,
            tc.tile_pool(name="ph", bufs=3, space="PSUM") as php,
            tc.tile_pool(name="pm", bufs=1, space="PSUM") as pmp,
        ):
            # --- input / weight DMAs (no on-chip casts needed: host sends
            # bf16).  Layer-0 needs XS3 + w0 + cb + wr + xb first; the big
            # conv block wc arrives during layer 0.
            XS3 = opool.tile([48, HW[0]], _F16, tag="XS3", name="XS3")
            for k in range(3):
                for h in range(2):
                    eng = nc.sync if (k * 2 + h) % 2 == 0 else nc.scalar
                    eng.dma_start(
                        XS3[k * 16 + h * 8 : k * 16 + h * 8 + 8, :],
                        d_x[:, h * HW[0] + k : h * HW[0] + k + HW[0]],
                    )
            W0T = wpool.tile([48, 128], _F16, tag="W0T", name="W0T")
            nc.scalar.dma_start(W0T[:, :], d_w0[:, :])
            CBT = wpool.tile([128, N_LAYERS], _F32, tag="CBT", name="CBT")
            nc.scalar.dma_start(CBT[:, :], d_cb[:, :])
            WRT = wpool.tile([128, 9 * 128], _F16, tag="WRT", name="WRT")
            nc.sync.dma_start(WRT[:, :], d_wr[:, :])
            XBT = wpool.tile([48, 128], _F16, tag="XBT", name="XBT")
            nc.scalar.dma_start(XBT[32:48, :], d_xb[:, :])
            WCT = wpool.tile([128, 8 * 3 * 128], _F16, tag="WCT", name="WCT")
            nc.gpsimd.dma_start(WCT[:, :], d_wc[:, :])
            W9T = wpool.tile([64, 192], _F16, tag="W9T", name="W9T")
            nc.gpsimd.dma_start(W9T[:, :], d_w9[:, :])
            WMH = wpool.tile([128, 80], _F16, tag="WMH", name="WMH")
            nc.gpsimd.dma_start(WMH[64:128, :], d_wm[:, :])
            WM9 = wpool.tile([64, 8], _F16, tag="WM9", name="WM9")
            nc.gpsimd.dma_start(WM9[:, :], d_wm9[:, :])
            MBT = wpool.tile([8, 1], _F32, tag="MBT", name="MBT")
            nc.gpsimd.dma_start(MBT[:, :], d_mb[:, :])

            pm = pmp.tile([8, FRAME], _F32, tag="pm", name="pm")

            # A[i]: fp16 matmul input of layer i (fold-i layout, i=1..8:
            # [128, HW[i]+2d]; layer 9 unfolded [64, 1152]).  H[i]: fp32
            # h_i in fold-(i-1) layout [128, HW[i-1]].
            A = [None] * N_LAYERS
            H = [None] * N_LAYERS

            for i in range(N_LAYERS):
                d = DIL[i]
                folded = i < 9
                hw = HW[i] if folded else W_Y[9]
                prows = 128 if folded else 64
                tl = _tiles(hw)
                d1 = DIL[i + 1] if i < 9 else 0
                if i < 9:
                    # allocate next layer's tensors
                    H[i + 1] = hpool.tile([128, hw], _F32, tag="H", name=f"H{i+1}")
                    if i < 8:
                        A[i + 1] = apool.tile(
                            [128, HW[i + 1] + 2 * d1], _F16, tag="A", name=f"A{i+1}"
                        )
                    else:
                        A[9] = apool.tile([64, W_H[9]], _F16, tag="A", name="A9")
                if 1 <= i <= 8:
                    # scratch for the fold-boundary TT piece: H[i] rows
                    # 64:128 cols [0,d) shifted down to rows 0:64
                    SCR = spool.tile([64, d], _F32, tag="SCR", name=f"SCR{i}")
                    nc.vector.tensor_copy(SCR[:, :], H[i][64:128, 0:d])
                cast2_done = False

                for ti, (j0, n) in enumerate(tl):
                    je = j0 + n
                    py = pyp.tile([prows, n], _F32, tag="py", name=f"py_{i}_{j0}")
                    # --- conv ---
                    if i == 0:
                        nc.tensor.matmul(
                            py[:, :], W0T[:, :], XS3[:, j0:je], start=True, stop=True
                        )
                    elif i < 9:
                        c0 = (i - 1) * 3 * 128
                        for k in range(3):
                            nc.tensor.matmul(
                                py[:, :],
                                WCT[:, c0 + k * 128 : c0 + (k + 1) * 128],
                                A[i][:, k * d + j0 : k * d + je],
                                start=(k == 0),
                                stop=(k == 2),
                            )
                    else:
                        for k in range(3):
                            nc.tensor.matmul(
                                py[:, :],
                                W9T[:, k * 64 : (k + 1) * 64],
                                A[9][:, k * d + j0 : k * d + je],
                                start=(k == 0),
                                stop=(k == 2),
                            )
                    # --- relu + bias ---
                    yt = ypool.tile([prows, n], _F16, tag="Y", name=f"Y_{i}_{j0}")
                    nc.scalar.activation(
                        yt[:, :], py[:, :], _RELU, bias=CBT[0:prows, i : i + 1]
                    )
                    # --- mixer (last tile; folded reads half-2 rows) ---
                    if ti == len(tl) - 1:
                        if folded:
                            nc.tensor.matmul(
                                pm[:, :],
                                WMH[64:128, i * 8 : (i + 1) * 8],
                                yt[64:128, n - FRAME : n],
                                start=(i == 0),
                                stop=False,
                                skip_group_check=True,
                            )
                        else:
                            nc.tensor.matmul(
                                pm[:, :],
                                WM9[:, :],
                                yt[:, :],
                                start=False,
                                stop=True,
                                skip_group_check=True,
                            )
                    if i == 9:
                        continue
                    # --- residual matmul -> ph (h_{i+1} increment) ---
                    ph = php.tile([128, n], _F32, tag="ph", name=f"ph_{i}_{j0}")
                    nc.tensor.matmul(
                        ph[:, :],
                        WRT[:, i * 128 : (i + 1) * 128],
                        yt[:, :],
                        start=True,
                        stop=(i != 0),
                    )
                    if i == 0:
                        nc.tensor.matmul(
                            ph[:, :],
                            XBT[32:48, :],
                            XS3[32:48, j0:je],
                            start=False,
                            stop=True,
                        )
                    # --- h chain: H[i+1] (fold-i) = ph + H[i] (DVE) ---
                    if i == 0:
                        nc.vector.tensor_copy(H[1][:, j0:je], ph[:, :])
                    else:
                        # rows 64:128 (half 2): aligned full width
                        nc.vector.tensor_add(
                            H[i + 1][64:128, j0:je],
                            ph[64:128, :],
                            H[i][64:128, d + j0 : d + je],
                        )
                        # rows 0:64 (half 1): crosses H[i]'s fold at hw-d
                        sa = min(je, hw - d)
                        if j0 < sa:
                            nc.vector.tensor_add(
                                H[i + 1][0:64, j0:sa],
                                ph[0:64, 0 : sa - j0],
                                H[i][0:64, 2 * d + j0 : 2 * d + sa],
                            )
                        sb = max(j0, hw - d)
                        if sb < je:
                            nc.vector.tensor_add(
                                H[i + 1][0:64, sb:je],
                                ph[0:64, sb - j0 : n],
                                SCR[:, sb - (hw - d) : je - (hw - d)],
                            )
                    # --- refold cast H[i+1] -> A[i+1] bf16 ---
                    if i < 8:
                        # A' fold-(i+1): rows0:64 = h[c], rows64:128 =
                        # h[HW[i+1]+c]; h stored fold-i in H[i+1]
                        nc.gpsimd.tensor_copy(
                            A[i + 1][0:64, j0:je], H[i + 1][0:64, j0:je]
                        )
                        nc.gpsimd.tensor_copy(
                            A[i + 1][64:128, d1 + j0 : d1 + je],
                            H[i + 1][64:128, j0:je],
                        )
                        if not cast2_done and je >= d1:
                            nc.vector.tensor_copy(
                                A[i + 1][0:64, hw : hw + d1],
                                H[i + 1][64:128, 0:d1],
                            )
                            cast2_done = True
                        if je == hw:  # last tile: half-2 head from half-1 tail
                            nc.vector.tensor_copy(
                                A[i + 1][64:128, 0:d1],
                                H[i + 1][0:64, hw - d1 : hw],
                            )
                    else:
                        # unfold into A9 [64, 1152]
                        nc.gpsimd.tensor_copy(A[9][:, j0:je], H[9][0:64, j0:je])
                        nc.vector.tensor_copy(
                            A[9][:, hw + j0 : hw + je], H[9][64:128, j0:je]
                        )

            out_sb = opool.tile([8, FRAME], _F32, tag="osb", name="osb")
            nc.scalar.activation(out_sb[:, :], pm[:, :], _IDENT, bias=MBT[:, 0:1])
            nc.sync.dma_start(d_out[:, :], out_sb[:, :])

    nc.compile()
    return nc


def _host_weights(c0_kernel, c_kernels, c_biases, io_kernels, io_biases,
                  mixer_kernel, mixer_bias):
    """Block-diagonal bf16 weights + io-bias folding, shared by all cores."""
    f16 = np.float16
    eye8 = np.eye(8, dtype=np.float32)
    eye16 = np.eye(16, dtype=np.float32)
    # layer-0: all 3 taps x 2 fold-halves in one K=48 stationary [48, 128]
    w0 = np.zeros((48, 128), dtype=np.float32)
    for k in range(3):
        blk = np.kron(eye8, c0_kernel[k, 0, :][None, :])  # [8, 64]
        for h in range(2):
            w0[k * 16 + h * 8 : k * 16 + h * 8 + 8, h * 64 : (h + 1) * 64] = blk
    # layers 1..8 folded taps [128, 8*3*128]
    wc = np.zeros((128, 8 * 3 * 128), dtype=np.float32)
    for li in range(1, 9):
        for k in range(3):
            wc[:, ((li - 1) * 3 + k) * 128 : ((li - 1) * 3 + k + 1) * 128] = (
                np.kron(eye16, c_kernels[li - 1, k])
            )
    # layer 9 unfolded taps [64, 192]
    w9 = np.concatenate(
        [np.kron(eye8, c_kernels[8, k]) for k in range(3)], axis=1
    ).astype(np.float32)
    # residual 1x1 convs [128, 9*128]
    wr = np.concatenate(
        [np.kron(eye16, io_kernels[i, 0]) for i in range(9)], axis=1
    ).astype(np.float32)
    # x broadcast for layer 0 residual [16, 128]
    xb = np.zeros((16, 128), dtype=np.float32)
    for h in range(2):
        xb[h * 8 : (h + 1) * 8, h * 64 : (h + 1) * 64] = np.kron(
            eye8, np.ones((1, 8), np.float32)
        )
    # mixer [64, 80] (layers 0..9; layer 9 block also sent separately)
    wm = np.concatenate(
        [
            np.kron(eye8, mixer_kernel[0, i * 8 : (i + 1) * 8, 0][:, None])
            for i in range(N_LAYERS)
        ],
        axis=1,
    ).astype(np.float32)
    # conv biases with io biases folded through the conv taps
    cb = np.zeros((8, N_LAYERS), dtype=np.float64)
    kappa = np.zeros(8, dtype=np.float64)
    for i in range(N_LAYERS):
        if i == 0:
            adj = np.zeros(8)
        else:
            adj = np.einsum("kio,i->o", c_kernels[i - 1].astype(np.float64),
                            kappa)
        cb[:, i] = c_biases[i].astype(np.float64) + adj
        if i < N_LAYERS - 1:
            kappa = kappa + io_biases[i].astype(np.float64)
    cb = np.tile(cb.astype(np.float32), (16, 1))  # [128, 10]
    mb = np.full((8, 1), float(np.asarray(mixer_bias).reshape(-1)[0]), np.float32)
    return dict(
        w0=w0.astype(f16), wc=wc.astype(f16), w9=w9.astype(f16),
        wr=wr.astype(f16), xb=xb.astype(f16), wm=wm.astype(f16),
        wm9=wm[:, 72:80].astype(f16), cb=cb, mb=mb,
    )


_NC_CACHE = None


def _get_nc():
    global _NC_CACHE
    if _NC_CACHE is None:
        _NC_CACHE = _build_program()
    return _NC_CACHE


def run(inputs, trace=False, **spmd_kwargs):
    """Run on 8 cores; returns (full output [64,128], BassKernelResults)."""
    x = np.asarray(inputs["x"], dtype=np.float32)
    shared = _host_weights(
        np.asarray(inputs["c0_kernel"], np.float32),
        np.asarray(inputs["c_kernels"], np.float32),
        np.asarray(inputs["c_biases"], np.float32),
        np.asarray(inputs["io_kernels"], np.float32),
        np.asarray(inputs["io_biases"], np.float32),
        np.asarray(inputs["mixer_kernel"], np.float32),
        np.asarray(inputs["mixer_bias"], np.float32),
    )
    xw = np.ascontiguousarray(x[:, T - W_X :]).astype(np.float16)
    in_maps = []
    for c in range(N_CORES):
        m = dict(shared)
        m["xw"] = np.ascontiguousarray(xw[c * B_LOC : (c + 1) * B_LOC])
        in_maps.append(m)
    nc = _get_nc()
    res = run_bass_kernel_spmd(
        nc, in_maps, core_ids=list(range(N_CORES)), trace=trace, **spmd_kwargs
    )
    out = np.concatenate([res.results[c]["out"] for c in range(N_CORES)], axis=0)
    return out.astype(np.float32), res


def kernel(**inputs):
    out, _ = run(inputs, trace=False)
    return out


# revision 5
# speedup vs baseline: 1.4730x; 1.4730x over previous
"""Trainium2 Bass kernel for nn_AudioDeviceModel (dilated causal conv stack).

Strategy (v3 — fp16 matmuls + time-folding + fp16-carried residual chain):
  - Data parallel: batch 64 sharded as 8 rows per core across 8 cores.
  - Only the last FRAME=128 timesteps are output; receptive field 2047, so
    only the last 2174 input samples matter.  Per-layer output windows W_Y
    shrink accordingly.
  - All matmul operands are fp16.  (float32r lowers to fp32_mode=HIGH on
    this toolchain: 4 cycles/row + a 219ns weight reload per matmul; fp16
    runs at 1 cycle/row with a cheap separate LDWEIGHTS.)  PSUM stays fp32.
  - Time folding: each folded layer splits its output window into two
    halves stacked on partitions (rows 0:64 = (b,c) of half 1, rows 64:128
    = half 2) with block-diagonal weights kron(eye16, W).  Each conv tap is
    one K=128 matmul over W_Y/2 columns (3 taps), the 1x1 residual is one
    K=128 matmul.  PE cost 2*W_Y cycles/layer vs 3*W_Y for tap-stacking,
    and every elementwise op covers all 128 partitions.
  - Layer 0 folds all 3 taps AND both halves into one K=48 matmul from a
    shifted-triplicated x tile (XS3); its residual broadcast of x reuses
    XS3 rows 32:48 via a ones block-matrix.  Layer 9 (only 128 output
    cols) runs unfolded with K=64 taps.
  - h chain is carried in fp16 inside the A tensors themselves (emulated
    end-to-end rel err ~1e-3 vs the 2e-2 gate; PSUM accumulation is fp32
    so only per-layer fp16 rounding enters).  Epilogue per layer:
      DVE:  drain ph (PSUM fp32) -> phs (SBUF fp16) per tile
      Pool: A[i+1] = phs + A[i] re-fold adds (all-fp16, SBUF-only,
            partition-base-aligned big segments)
      DVE:  the two small fold-boundary segments (partition-crossing)
      ACT:  relu+bias -> yt fp16
  - Mixer: 10 accumulated [64,8]x[64,128] fp16 matmuls interleaved at each
    layer's end (skip_group_check) + bias.
"""

import sys

import numpy as np

try:
    import concourse.bass as bass
except ImportError:  # fresh environment without the site path
    sys.path.insert(0, "/opt/trn_rl_repo")
    import concourse.bass as bass

import concourse.tile as tile
from concourse import bacc, mybir
from concourse.bass_utils import run_bass_kernel_spmd

N_LAYERS = 10
FRAME = 128
B, T = 64, 4096
N_CORES = 8
B_LOC = B // N_CORES  # 8 batch rows per core
NT = 512  # time-tile (one PSUM bank of f32)

# per-layer dilations and windows
DIL = [2**i for i in range(N_LAYERS)]
W_Y = [0] * N_LAYERS  # output window of layer i
W_H = [0] * N_LAYERS  # input window of layer i
W_Y[N_LAYERS - 1] = FRAME
for _i in range(N_LAYERS - 1, -1, -1):
    W_H[_i] = W_Y[_i] + 2 * DIL[_i]
    if _i > 0:
        W_Y[_i - 1] = W_H[_i]
W_X = W_H[0]  # 2174
HW = [w // 2 for w in W_Y]  # folded half-width (layers 0..8)

_F32 = mybir.dt.float32
_F16 = mybir.dt.float16
_RELU = mybir.ActivationFunctionType.Relu
_IDENT = mybir.ActivationFunctionType.Identity


def _tiles(wy):
    """End-aligned tiling: ragged first tile, then 512-wide tiles."""
    r = wy % NT
    starts = ([0] if r else []) + list(range(r, wy, NT))
    return [(s, (starts[k + 1] if k + 1 < len(starts) else wy) - s)
            for k, s in enumerate(starts)]


def _build_program():
    nc = bacc.Bacc(
        "TRN2",
        target_bir_lowering=False,
        debug=False,
        enable_asserts=True,
        num_devices=N_CORES,
    )

    d_x = nc.dram_tensor("xw", [B_LOC, W_X], _F16, kind="ExternalInput").ap()
    d_w0 = nc.dram_tensor("w0", [48, 128], _F16, kind="ExternalInput").ap()
    d_wc = nc.dram_tensor("wc", [128, 8 * 3 * 128], _F16, kind="ExternalInput").ap()
    d_w9 = nc.dram_tensor("w9", [64, 192], _F16, kind="ExternalInput").ap()
    d_wr = nc.dram_tensor("wr", [128, 9 * 128], _F16, kind="ExternalInput").ap()
    d_xb = nc.dram_tensor("xb", [16, 128], _F16, kind="ExternalInput").ap()
    d_wm = nc.dram_tensor("wm", [64, 80], _F16, kind="ExternalInput").ap()
    d_wm9 = nc.dram_tensor("wm9", [64, 8], _F16, kind="ExternalInput").ap()
    d_cb = nc.dram_tensor("cb", [128, N_LAYERS], _F32, kind="ExternalInput").ap()
    d_mb = nc.dram_tensor("mb", [8, 1], _F32, kind="ExternalInput").ap()
    d_out = nc.dram_tensor("out", [B_LOC, FRAME], _F32, kind="ExternalOutput").ap()

    with tile.TileContext(nc) as tc:
        with (
            tc.tile_pool(name="wpool", bufs=1) as wpool,
            tc.tile_pool(name="apool", bufs=2) as apool,
            tc.tile_pool(name="ypool", bufs=4) as ypool,
            tc.tile_pool(name="spool", bufs=3) as spool,
            tc.tile_pool(name="opool", bufs=1) as opool,
            tc.tile_pool(name="py", bufs=3, space="PSUM") as pyp,
            tc.tile_pool(name="ph", bufs=3, space="PSUM") as php,
            tc.tile_pool(name="pm", bufs=1, space="PSUM") as pmp,
        ):
            # --- input / weight DMAs (host sends fp16; no on-chip casts).
            # Layer-0 needs XS3 + w0 + cb + wr + xb first; the big conv
            # block wc arrives during layer 0.
            XS3 = opool.tile([48, HW[0]], _F16, tag="XS3", name="XS3")
            for k in range(3):
                for h in range(2):
                    eng = nc.sync if (k * 2 + h) % 2 == 0 else nc.scalar
                    eng.dma_start(
                        XS3[k * 16 + h * 8 : k * 16 + h * 8 + 8, :],
                        d_x[:, h * HW[0] + k : h * HW[0] + k + HW[0]],
                    )
            W0T = wpool.tile([48, 128], _F16, tag="W0T", name="W0T")
            nc.scalar.dma_start(W0T[:, :], d_w0[:, :])
            CBT = wpool.tile([128, N_LAYERS], _F32, tag="CBT", name="CBT")
            nc.scalar.dma_start(CBT[:, :], d_cb[:, :])
            WRT = wpool.tile([128, 9 * 128], _F16, tag="WRT", name="WRT")
            nc.sync.dma_start(WRT[:, :], d_wr[:, :])
            XBT = wpool.tile([48, 128], _F16, tag="XBT", name="XBT")
            nc.scalar.dma_start(XBT[32:48, :], d_xb[:, :])
            WCT = wpool.tile([128, 8 * 3 * 128], _F16, tag="WCT", name="WCT")
            nc.gpsimd.dma_start(WCT[:, :], d_wc[:, :])
            W9T = wpool.tile([64, 192], _F16, tag="W9T", name="W9T")
            nc.gpsimd.dma_start(W9T[:, :], d_w9[:, :])
            WMH = wpool.tile([128, 80], _F16, tag="WMH", name="WMH")
            nc.gpsimd.dma_start(WMH[64:128, :], d_wm[:, :])
            WM9 = wpool.tile([64, 8], _F16, tag="WM9", name="WM9")
            nc.gpsimd.dma_start(WM9[:, :], d_wm9[:, :])
            MBT = wpool.tile([8, 1], _F32, tag="MBT", name="MBT")
            nc.gpsimd.dma_start(MBT[:, :], d_mb[:, :])

            pm = pmp.tile([8, FRAME], _F32, tag="pm", name="pm")

            # A[i]: fp16 h_i in fold-i layout (i=1..8: [128, HW[i]+2d];
            # layer 9 unfolded [64, 1152]).  Carries the residual chain.
            A = [None] * N_LAYERS

            for i in range(N_LAYERS):
                d = DIL[i]
                folded = i < 9
                hw = HW[i] if folded else W_Y[9]
                prows = 128 if folded else 64
                tl = _tiles(hw)
                d1 = DIL[i + 1] if i < 9 else 0
                if i < 8:
                    A[i + 1] = apool.tile(
                        [128, HW[i + 1] + 2 * d1], _F16, tag="A", name=f"A{i+1}"
                    )
                elif i == 8:
                    A[9] = apool.tile([64, W_H[9]], _F16, tag="A", name="A9")

                for ti, (j0, n) in enumerate(tl):
                    je = j0 + n
                    last = ti == len(tl) - 1
                    py = pyp.tile([prows, n], _F32, tag="py", name=f"py_{i}_{j0}")
                    # --- conv ---
                    if i == 0:
                        nc.tensor.matmul(
                            py[:, :], W0T[:, :], XS3[:, j0:je], start=True, stop=True
                        )
                    elif i < 9:
                        c0 = (i - 1) * 3 * 128
                        for k in range(3):
                            nc.tensor.matmul(
                                py[:, :],
                                WCT[:, c0 + k * 128 : c0 + (k + 1) * 128],
                                A[i][:, k * d + j0 : k * d + je],
                                start=(k == 0),
                                stop=(k == 2),
                            )
                    else:
                        for k in range(3):
                            nc.tensor.matmul(
                                py[:, :],
                                W9T[:, k * 64 : (k + 1) * 64],
                                A[9][:, k * d + j0 : k * d + je],
                                start=(k == 0),
                                stop=(k == 2),
                            )
                    # --- relu + bias ---
                    yt = ypool.tile([prows, n], _F16, tag="Y", name=f"Y_{i}_{j0}")
                    nc.scalar.activation(
                        yt[:, :], py[:, :], _RELU, bias=CBT[0:prows, i : i + 1]
                    )
                    # --- mixer (last tile; folded reads half-2 rows) ---
                    if last:
                        if folded:
                            nc.tensor.matmul(
                                pm[:, :],
                                WMH[64:128, i * 8 : (i + 1) * 8],
                                yt[64:128, n - FRAME : n],
                                start=(i == 0),
                                stop=False,
                                skip_group_check=True,
                            )
                        else:
                            nc.tensor.matmul(
                                pm[:, :],
                                WM9[:, :],
                                yt[:, :],
                                start=False,
                                stop=True,
                                skip_group_check=True,
                            )
                    if i == 9:
                        continue
                    # --- residual matmul -> ph = U_i y (+ x broadcast) ---
                    ph = php.tile([128, n], _F32, tag="ph", name=f"ph_{i}_{j0}")
                    nc.tensor.matmul(
                        ph[:, :],
                        WRT[:, i * 128 : (i + 1) * 128],
                        yt[:, :],
                        start=True,
                        stop=(i != 0),
                    )
                    if i == 0:
                        nc.tensor.matmul(
                            ph[:, :],
                            XBT[32:48, :],
                            XS3[32:48, j0:je],
                            start=False,
                            stop=True,
                        )
                    # --- drain PSUM -> fp16 (DVE) ---
                    phs = spool.tile([128, n], _F16, tag="PHS", name=f"phs_{i}_{j0}")
                    nc.vector.tensor_copy(phs[:, :], ph[:, :])
                    # --- re-fold adds: A[i+1] = phs + A[i] (fold-i aligned
                    # inputs, fold-(i+1) output).  Big same-base segments on
                    # Pool, chunked per tile; small crossing ones on DVE.
                    if i == 0:
                        nc.gpsimd.tensor_copy(A[1][0:64, j0:je], phs[0:64, :])
                        nc.gpsimd.tensor_copy(
                            A[1][64:128, d1 + j0 : d1 + je], phs[64:128, :]
                        )
                    elif i < 8:
                        nc.gpsimd.tensor_add(
                            A[i + 1][0:64, j0:je],
                            phs[0:64, :],
                            A[i][0:64, 2 * d + j0 : 2 * d + je],
                        )
                        nc.gpsimd.tensor_add(
                            A[i + 1][64:128, d1 + j0 : d1 + je],
                            phs[64:128, :],
                            A[i][64:128, 2 * d + j0 : 2 * d + je],
                        )
                    else:
                        # unfold into A9 [64, 1152]
                        nc.gpsimd.tensor_add(
                            A[9][:, j0:je],
                            phs[0:64, :],
                            A[8][0:64, 2 * d + j0 : 2 * d + je],
                        )
                        nc.vector.tensor_add(
                            A[9][:, hw + j0 : hw + je],
                            phs[64:128, :],
                            A[8][64:128, 2 * d + j0 : 2 * d + je],
                        )
                    # small fold-boundary segments (first/last tile, DVE)
                    if i < 8:
                        if ti == 0:
                            if i == 0:
                                nc.vector.tensor_copy(
                                    A[1][0:64, hw : hw + d1], phs[64:128, 0:d1]
                                )
                            else:
                                nc.vector.tensor_add(
                                    A[i + 1][0:64, hw : hw + d1],
                                    phs[64:128, 0:d1],
                                    A[i][64:128, 2 * d : 2 * d + d1],
                                )
                        if last:
                            co = n - d1  # = (hw - d1) - j0 on the last tile
                            if i == 0:
                                nc.vector.tensor_copy(
                                    A[1][64:128, 0:d1], phs[0:64, co:n]
                                )
                            else:
                                nc.vector.tensor_add(
                                    A[i + 1][64:128, 0:d1],
                                    phs[0:64, co:n],
                                    A[i][0:64, 2 * d + hw - d1 : 2 * d + hw],
                                )

            out_sb = opool.tile([8, FRAME], _F32, tag="osb", name="osb")
            nc.scalar.activation(out_sb[:, :], pm[:, :], _IDENT, bias=MBT[:, 0:1])
            nc.sync.dma_start(d_out[:, :], out_sb[:, :])

    nc.compile()
    return nc


def _host_weights(c0_kernel, c_kernels, c_biases, io_kernels, io_biases,
                  mixer_kernel, mixer_bias):
    """Block-diagonal fp16 weights + io-bias folding, shared by all cores."""
    f16 = np.float16
    eye8 = np.eye(8, dtype=np.float32)
    eye16 = np.eye(16, dtype=np.float32)
    # layer-0: all 3 taps x 2 fold-halves in one K=48 stationary [48, 128]
    w0 = np.zeros((48, 128), dtype=np.float32)
    for k in range(3):
        blk = np.kron(eye8, c0_kernel[k, 0, :][None, :])  # [8, 64]
        for h in range(2):
            w0[k * 16 + h * 8 : k * 16 + h * 8 + 8, h * 64 : (h + 1) * 64] = blk
    # layers 1..8 folded taps [128, 8*3*128]
    wc = np.zeros((128, 8 * 3 * 128), dtype=np.float32)
    for li in range(1, 9):
        for k in range(3):
            wc[:, ((li - 1) * 3 + k) * 128 : ((li - 1) * 3 + k + 1) * 128] = (
                np.kron(eye16, c_kernels[li - 1, k])
            )
    # layer 9 unfolded taps [64, 192]
    w9 = np.concatenate(
        [np.kron(eye8, c_kernels[8, k]) for k in range(3)], axis=1
    ).astype(np.float32)
    # residual 1x1 convs [128, 9*128]
    wr = np.concatenate(
        [np.kron(eye16, io_kernels[i, 0]) for i in range(9)], axis=1
    ).astype(np.float32)
    # x broadcast for layer 0 residual [16, 128]
    xb = np.zeros((16, 128), dtype=np.float32)
    for h in range(2):
        xb[h * 8 : (h + 1) * 8, h * 64 : (h + 1) * 64] = np.kron(
            eye8, np.ones((1, 8), np.float32)
        )
    # mixer [64, 80] (layers 0..9; layer 9 block also sent separately)
    wm = np.concatenate(
        [
            np.kron(eye8, mixer_kernel[0, i * 8 : (i + 1) * 8, 0][:, None])
            for i in range(N_LAYERS)
        ],
        axis=1,
    ).astype(np.float32)
    # conv biases with io biases folded through the conv taps
    cb = np.zeros((8, N_LAYERS), dtype=np.float64)
    kappa = np.zeros(8, dtype=np.float64)
    for i in range(N_LAYERS):
        if i == 0:
            adj = np.zeros(8)
        else:
            adj = np.einsum("kio,i->o", c_kernels[i - 1].astype(np.float64),
                            kappa)
        cb[:, i] = c_biases[i].astype(np.float64) + adj
        if i < N_LAYERS - 1:
            kappa = kappa + io_biases[i].astype(np.float64)
    cb = np.tile(cb.astype(np.float32), (16, 1))  # [128, 10]
    mb = np.full((8, 1), float(np.asarray(mixer_bias).reshape(-1)[0]), np.float32)
    return dict(
        w0=w0.astype(f16), wc=wc.astype(f16), w9=w9.astype(f16),
        wr=wr.astype(f16), xb=xb.astype(f16), wm=wm.astype(f16),
        wm9=wm[:, 72:80].astype(f16), cb=cb, mb=mb,
    )


_NC_CACHE = None


def _get_nc():
    global _NC_CACHE
    if _NC_CACHE is None:
        _NC_CACHE = _build_program()
    return _NC_CACHE


def run(inputs, trace=False, **spmd_kwargs):
    """Run on 8 cores; returns (full output [64,128], BassKernelResults)."""
    x = np.asarray(inputs["x"], dtype=np.float32)
    shared = _host_weights(
        np.asarray(inputs["c0_kernel"], np.float32),
        np.asarray(inputs["c_kernels"], np.float32),
        np.asarray(inputs["c_biases"], np.float32),
        np.asarray(inputs["io_kernels"], np.float32),
        np.asarray(inputs["io_biases"], np.float32),
        np.asarray(inputs["mixer_kernel"], np.float32),
        np.asarray(inputs["mixer_bias"], np.float32),
    )
    xw = np.ascontiguousarray(x[:, T - W_X :]).astype(np.float16)
    in_maps = []
    for c in range(N_CORES):
        m = dict(shared)
        m["xw"] = np.ascontiguousarray(xw[c * B_LOC : (c + 1) * B_LOC])
        in_maps.append(m)
    nc = _get_nc()
    res = run_bass_kernel_spmd(
        nc, in_maps, core_ids=list(range(N_CORES)), trace=trace, **spmd_kwargs
    )
    out = np.concatenate([res.results[c]["out"] for c in range(N_CORES)], axis=0)
    return out.astype(np.float32), res


def kernel(**inputs):
    out, _ = run(inputs, trace=False)
    return out


# revision 7
# speedup vs baseline: 1.6526x; 1.1220x over previous
"""Trainium2 Bass kernel for nn_AudioDeviceModel (dilated causal conv stack).

Strategy (v4 — fp16 matmuls + time-folding + fp16-carried residual chain):
  - Data parallel: batch 64 sharded as 8 rows per core across 8 cores.
  - Only the last FRAME=128 timesteps are output; receptive field 2047, so
    only the last 2174 input samples matter.  Per-layer output windows W_Y
    shrink accordingly.
  - All matmul operands are fp16.  (float32r lowers to fp32_mode=HIGH on
    this toolchain: 4 cycles/row + a 219ns weight reload per matmul; fp16
    runs at 1 cycle/row with a cheap separate LDWEIGHTS.)  PSUM stays fp32.
  - Time folding: each folded layer splits its output window into two
    halves stacked on partitions (rows 0:64 = (b,c) of half 1, rows 64:128
    = half 2) with block-diagonal weights kron(eye16, W).  Each conv tap is
    one K=128 matmul over W_Y/2 columns (3 taps), the 1x1 residual is one
    K=128 matmul.  PE cost 2*W_Y cycles/layer vs 3*W_Y for tap-stacking.
  - Layer 0 folds all 3 taps AND both halves into one K=48 matmul from a
    shifted-triplicated x tile (XS3, loaded by ONE 4-level-AP DMA); its
    residual broadcast of x reuses XS3 rows 32:48 via a ones block-matrix.
    Layer 9 (only 128 output cols) runs unfolded with K=64 taps.
  - Prologue: all fp16 weights live in ONE packed [128, 4760] DRAM tensor
    split across the two HWDGE rings (sync+scalar), fp32 biases in one
    [128, 11] tensor.  No SWDGE (gpsimd) DMAs — their ~2us fixed cost each
    serialized ~10us of prologue in v3.
  - h chain is carried in fp16 inside the A tensors themselves (emulated
    end-to-end rel err ~1e-3 vs the 2e-2 gate; PSUM accumulation is fp32).
    Epilogue per layer, balanced across engines by measured rates
    (ACT 1.09/DVE 1.16/Pool 1.93 ns per col incl per-inst overhead):
      relu+bias -> yt fp16:            ACT per tile
      drain ph (PSUM) -> phs fp16:     ACT for 512-tiles 0.., DVE middle
      seg1 re-fold add (rows 0:64):    DVE per tile
      seg4 re-fold add (rows 64:128):  Pool for 512-tiles, DVE ragged
      fold-boundary small segments:    DVE
  - Mixer: 10 accumulated [64,8]x[64,128] fp16 matmuls interleaved at each
    layer's end (skip_group_check) + bias.
"""

import sys

import numpy as np

try:
    import concourse.bass as bass
except ImportError:  # fresh environment without the site path
    sys.path.insert(0, "/opt/trn_rl_repo")
    import concourse.bass as bass

import concourse.tile as tile
from concourse import bacc, mybir
from concourse.bass_utils import run_bass_kernel_spmd

N_LAYERS = 10
FRAME = 128
B, T = 64, 4096
N_CORES = 8
B_LOC = B // N_CORES  # 8 batch rows per core
NT = 512  # time-tile (one PSUM bank of f32)

# per-layer dilations and windows
DIL = [2**i for i in range(N_LAYERS)]
W_Y = [0] * N_LAYERS  # output window of layer i
W_H = [0] * N_LAYERS  # input window of layer i
W_Y[N_LAYERS - 1] = FRAME
for _i in range(N_LAYERS - 1, -1, -1):
    W_H[_i] = W_Y[_i] + 2 * DIL[_i]
    if _i > 0:
        W_Y[_i - 1] = W_H[_i]
W_X = W_H[0]  # 2174
HW = [w // 2 for w in W_Y]  # folded half-width (layers 0..8)

# packed fp16 weight tensor column offsets
C_W0 = 0          # [48, 128]   layer-0 stacked taps
C_WC = 128        # [128, 3072] layers 1..8 folded taps
C_W9 = 3200       # [64, 192]   layer-9 taps
C_WR = 3392       # [128, 1152] residual 1x1 blocks
C_XB = 4544       # [16, 128]   x broadcast (stored at rows 32:48)
C_WM = 4672       # [64, 80]    mixer (stored at rows 64:128)
C_WM9 = 4752      # [64, 8]     layer-9 mixer (rows 0:64)
WGT_COLS = 4760

_F32 = mybir.dt.float32
_F16 = mybir.dt.float16
_RELU = mybir.ActivationFunctionType.Relu
_IDENT = mybir.ActivationFunctionType.Identity


def _tiles(wy):
    """End-aligned tiling: ragged first tile, then 512-wide tiles."""
    r = wy % NT
    starts = ([0] if r else []) + list(range(r, wy, NT))
    return [(s, (starts[k + 1] if k + 1 < len(starts) else wy) - s)
            for k, s in enumerate(starts)]


def _build_program():
    nc = bacc.Bacc(
        "TRN2",
        target_bir_lowering=False,
        debug=False,
        enable_asserts=True,
        num_devices=N_CORES,
    )

    d_x = nc.dram_tensor("xw", [B_LOC, W_X], _F16, kind="ExternalInput").ap()
    d_wgt = nc.dram_tensor("wgt", [128, WGT_COLS], _F16, kind="ExternalInput").ap()
    d_wf = nc.dram_tensor("wf", [128, 11], _F32, kind="ExternalInput").ap()
    d_out = nc.dram_tensor("out", [B_LOC, FRAME], _F32, kind="ExternalOutput").ap()

    with tile.TileContext(nc) as tc:
        with (
            tc.tile_pool(name="wpool", bufs=1) as wpool,
            tc.tile_pool(name="apool", bufs=2) as apool,
            tc.tile_pool(name="ypool", bufs=4) as ypool,
            tc.tile_pool(name="spool", bufs=3) as spool,
            tc.tile_pool(name="opool", bufs=1) as opool,
            tc.tile_pool(name="py", bufs=3, space="PSUM") as pyp,
            tc.tile_pool(name="ph", bufs=3, space="PSUM") as php,
            tc.tile_pool(name="pm", bufs=1, space="PSUM") as pmp,
        ):
            # --- prologue: HWDGE rings only (sync + scalar), ordered so
            # layer 0's operands land first.  No SWDGE (gpsimd) DMAs.
            XS3 = opool.tile([48, HW[0]], _F16, tag="XS3", name="XS3")
            WF = wpool.tile([128, 11], _F32, tag="WF", name="WF")
            WGT = wpool.tile([128, WGT_COLS], _F16, tag="WGT", name="WGT")
            nc.sync.dma_start(WGT[:, 0:128], d_wgt[:, 0:128])  # w0 first
            nc.scalar.dma_start(WF[:, :], d_wf[:, :])
            for k in range(3):
                for h in range(2):
                    eng = nc.sync if h == 0 else nc.scalar
                    eng.dma_start(
                        XS3[k * 16 + h * 8 : k * 16 + h * 8 + 8, :],
                        d_x[:, h * HW[0] + k : h * HW[0] + k + HW[0]],
                    )
            half = C_WR  # split: conv taps finish on sync, rest on scalar
            nc.sync.dma_start(WGT[:, 128:half], d_wgt[:, 128:half])
            nc.scalar.dma_start(WGT[:, half:], d_wgt[:, half:])

            pm = pmp.tile([8, FRAME], _F32, tag="pm", name="pm")

            # A[i]: fp16 h_i in fold-i layout (i=1..8: [128, HW[i]+2d];
            # layer 9 unfolded [64, 1152]).  Carries the residual chain.
            A = [None] * N_LAYERS

            for i in range(N_LAYERS):
                d = DIL[i]
                folded = i < 9
                hw = HW[i] if folded else W_Y[9]
                prows = 128 if folded else 64
                tl = _tiles(hw)
                d1 = DIL[i + 1] if i < 9 else 0
                if i < 8:
                    A[i + 1] = apool.tile(
                        [128, HW[i + 1] + 2 * d1], _F16, tag="A", name=f"A{i+1}"
                    )
                elif i == 8:
                    A[9] = apool.tile([64, W_H[9]], _F16, tag="A", name="A9")

                for ti, (j0, n) in enumerate(tl):
                    je = j0 + n
                    last = ti == len(tl) - 1
                    py = pyp.tile([prows, n], _F32, tag="py", name=f"py_{i}_{j0}")
                    # --- conv ---
                    if i == 0:
                        nc.tensor.matmul(
                            py[:, :], WGT[0:48, C_W0 : C_W0 + 128],
                            XS3[:, j0:je], start=True, stop=True,
                        )
                    elif i < 9:
                        c0 = C_WC + (i - 1) * 3 * 128
                        for k in range(3):
                            nc.tensor.matmul(
                                py[:, :],
                                WGT[:, c0 + k * 128 : c0 + (k + 1) * 128],
                                A[i][:, k * d + j0 : k * d + je],
                                start=(k == 0),
                                stop=(k == 2),
                            )
                    else:
                        for k in range(3):
                            nc.tensor.matmul(
                                py[:, :],
                                WGT[0:64, C_W9 + k * 64 : C_W9 + (k + 1) * 64],
                                A[9][:, k * d + j0 : k * d + je],
                                start=(k == 0),
                                stop=(k == 2),
                            )
                    # --- relu + bias ---
                    yt = ypool.tile([prows, n], _F16, tag="Y", name=f"Y_{i}_{j0}")
                    nc.scalar.activation(
                        yt[:, :], py[:, :], _RELU, bias=WF[0:prows, i : i + 1]
                    )
                    # --- mixer (last tile; folded reads half-2 rows) ---
                    if last:
                        if folded:
                            nc.tensor.matmul(
                                pm[:, :],
                                WGT[64:128, C_WM + i * 8 : C_WM + (i + 1) * 8],
                                yt[64:128, n - FRAME : n],
                                start=(i == 0),
                                stop=False,
                                skip_group_check=True,
                            )
                        else:
                            nc.tensor.matmul(
                                pm[:, :],
                                WGT[0:64, C_WM9 : C_WM9 + 8],
                                yt[:, :],
                                start=False,
                                stop=True,
                                skip_group_check=True,
                            )
                    if i == 9:
                        continue
                    # --- residual matmul -> ph = U_i y (+ x broadcast) ---
                    ph = php.tile([128, n], _F32, tag="ph", name=f"ph_{i}_{j0}")
                    nc.tensor.matmul(
                        ph[:, :],
                        WGT[:, C_WR + i * 128 : C_WR + (i + 1) * 128],
                        yt[:, :],
                        start=True,
                        stop=(i != 0),
                    )
                    if i == 0:
                        nc.tensor.matmul(
                            ph[:, :],
                            WGT[32:48, C_XB : C_XB + 128],
                            XS3[32:48, j0:je],
                            start=False,
                            stop=True,
                        )
                    # --- drain PSUM -> fp16 (ACT for the first 512-tile,
                    # DVE otherwise; per measured-rate balance) ---
                    phs = spool.tile([128, n], _F16, tag="PHS", name=f"phs_{i}_{j0}")
                    if n == NT and ti >= 1:
                        nc.scalar.copy(phs[:, :], ph[:, :])
                    else:
                        nc.vector.tensor_copy(phs[:, :], ph[:, :])
                    # --- re-fold adds: A[i+1] = phs + A[i] (fold-i aligned
                    # inputs, fold-(i+1) output) ---
                    if i == 0:
                        # h_1 = ph directly (copies, no add)
                        nc.vector.tensor_copy(A[1][0:64, j0:je], phs[0:64, :])
                        if n == NT:
                            nc.gpsimd.tensor_copy(
                                A[1][64:128, d1 + j0 : d1 + je], phs[64:128, :]
                            )
                        else:
                            nc.vector.tensor_copy(
                                A[1][64:128, d1 + j0 : d1 + je], phs[64:128, :]
                            )
                    elif i < 8:
                        nc.vector.tensor_add(
                            A[i + 1][0:64, j0:je],
                            phs[0:64, :],
                            A[i][0:64, 2 * d + j0 : 2 * d + je],
                        )
                        if n == NT:
                            nc.gpsimd.tensor_add(
                                A[i + 1][64:128, d1 + j0 : d1 + je],
                                phs[64:128, :],
                                A[i][64:128, 2 * d + j0 : 2 * d + je],
                            )
                        else:
                            nc.vector.tensor_add(
                                A[i + 1][64:128, d1 + j0 : d1 + je],
                                phs[64:128, :],
                                A[i][64:128, 2 * d + j0 : 2 * d + je],
                            )
                    else:
                        # unfold into A9 [64, 1152]
                        nc.vector.tensor_add(
                            A[9][:, j0:je],
                            phs[0:64, :],
                            A[8][0:64, 2 * d + j0 : 2 * d + je],
                        )
                        nc.vector.tensor_add(
                            A[9][:, hw + j0 : hw + je],
                            phs[64:128, :],
                            A[8][64:128, 2 * d + j0 : 2 * d + je],
                        )
                    # small fold-boundary segments (first/last tile, DVE)
                    if i < 8:
                        if ti == 0:
                            if i == 0:
                                nc.vector.tensor_copy(
                                    A[1][0:64, hw : hw + d1], phs[64:128, 0:d1]
                                )
                            else:
                                nc.vector.tensor_add(
                                    A[i + 1][0:64, hw : hw + d1],
                                    phs[64:128, 0:d1],
                                    A[i][64:128, 2 * d : 2 * d + d1],
                                )
                        if last:
                            co = n - d1  # = (hw - d1) - j0 on the last tile
                            if i == 0:
                                nc.vector.tensor_copy(
                                    A[1][64:128, 0:d1], phs[0:64, co:n]
                                )
                            else:
                                nc.vector.tensor_add(
                                    A[i + 1][64:128, 0:d1],
                                    phs[0:64, co:n],
                                    A[i][0:64, 2 * d + hw - d1 : 2 * d + hw],
                                )

            out_sb = opool.tile([8, FRAME], _F32, tag="osb", name="osb")
            nc.scalar.activation(
                out_sb[:, :], pm[:, :], _IDENT, bias=WF[0:8, 10:11]
            )
            nc.sync.dma_start(d_out[:, :], out_sb[:, :])

    nc.compile()
    return nc


def _host_weights(c0_kernel, c_kernels, c_biases, io_kernels, io_biases,
                  mixer_kernel, mixer_bias):
    """Packed block-diagonal fp16 weights + fp32 biases with io folding."""
    f16 = np.float16
    eye8 = np.eye(8, dtype=np.float32)
    eye16 = np.eye(16, dtype=np.float32)
    wgt = np.zeros((128, WGT_COLS), dtype=np.float32)
    # layer-0: all 3 taps x 2 fold-halves in one K=48 stationary [48, 128]
    for k in range(3):
        blk = np.kron(eye8, c0_kernel[k, 0, :][None, :])  # [8, 64]
        for h in range(2):
            wgt[k * 16 + h * 8 : k * 16 + h * 8 + 8,
                C_W0 + h * 64 : C_W0 + (h + 1) * 64] = blk
    # layers 1..8 folded taps
    for li in range(1, 9):
        for k in range(3):
            c = C_WC + ((li - 1) * 3 + k) * 128
            wgt[:, c : c + 128] = np.kron(eye16, c_kernels[li - 1, k])
    # layer 9 unfolded taps
    for k in range(3):
        wgt[0:64, C_W9 + k * 64 : C_W9 + (k + 1) * 64] = np.kron(
            eye8, c_kernels[8, k]
        )
    # residual 1x1 convs
    for i in range(9):
        wgt[:, C_WR + i * 128 : C_WR + (i + 1) * 128] = np.kron(
            eye16, io_kernels[i, 0]
        )
    # x broadcast for layer 0 residual (rows 32:48 to match XS3 tap-2 rows)
    for h in range(2):
        wgt[32 + h * 8 : 32 + (h + 1) * 8,
            C_XB + h * 64 : C_XB + (h + 1) * 64] = np.kron(
            eye8, np.ones((1, 8), np.float32)
        )
    # mixer (rows 64:128 so lhsT base matches the folded yt half-2 rows)
    wm = np.concatenate(
        [
            np.kron(eye8, mixer_kernel[0, i * 8 : (i + 1) * 8, 0][:, None])
            for i in range(N_LAYERS)
        ],
        axis=1,
    ).astype(np.float32)
    wgt[64:128, C_WM : C_WM + 80] = wm
    wgt[0:64, C_WM9 : C_WM9 + 8] = wm[:, 72:80]
    # conv biases with io biases folded through the conv taps
    cb = np.zeros((8, N_LAYERS), dtype=np.float64)
    kappa = np.zeros(8, dtype=np.float64)
    for i in range(N_LAYERS):
        if i == 0:
            adj = np.zeros(8)
        else:
            adj = np.einsum("kio,i->o", c_kernels[i - 1].astype(np.float64),
                            kappa)
        cb[:, i] = c_biases[i].astype(np.float64) + adj
        if i < N_LAYERS - 1:
            kappa = kappa + io_biases[i].astype(np.float64)
    wf = np.zeros((128, 11), dtype=np.float32)
    wf[:, 0:10] = np.tile(cb.astype(np.float32), (16, 1))
    wf[0:8, 10] = float(np.asarray(mixer_bias).reshape(-1)[0])
    return dict(wgt=wgt.astype(f16), wf=wf)


_NC_CACHE = None


def _get_nc():
    global _NC_CACHE
    if _NC_CACHE is None:
        _NC_CACHE = _build_program()
    return _NC_CACHE


def run(inputs, trace=False, **spmd_kwargs):
    """Run on 8 cores; returns (full output [64,128], BassKernelResults)."""
    x = np.asarray(inputs["x"], dtype=np.float32)
    shared = _host_weights(
        np.asarray(inputs["c0_kernel"], np.float32),
        np.asarray(inputs["c_kernels"], np.float32),
        np.asarray(inputs["c_biases"], np.float32),
        np.asarray(inputs["io_kernels"], np.float32),
        np.asarray(inputs["io_biases"], np.float32),
        np.asarray(inputs["mixer_kernel"], np.float32),
        np.asarray(inputs["mixer_bias"], np.float32),
    )
    xw = np.ascontiguousarray(x[:, T - W_X :]).astype(np.float16)
    in_maps = []
    for c in range(N_CORES):
        m = dict(shared)
        m["xw"] = np.ascontiguousarray(xw[c * B_LOC : (c + 1) * B_LOC])
        in_maps.append(m)
    nc = _get_nc()
    res = run_bass_kernel_spmd(
        nc, in_maps, core_ids=list(range(N_CORES)), trace=trace, **spmd_kwargs
    )
    out = np.concatenate([res.results[c]["out"] for c in range(N_CORES)], axis=0)
    return out.astype(np.float32), res


def kernel(**inputs):
    out, _ = run(inputs, trace=False)
    return out
